# revision 29
# baseline (speedup 1.0000x reference)
"""Trainium2 Bass kernel for ExllamaLinear (int4 group-quantized 4096x4096 linear).

out[b,s,o] = x @ W + bias,  W[i,o] = (nib4[i,o] - z[g(i),o]) * s[g(i),o]

Strategy (8 NeuronCores, data-parallel over tokens):
  - Host: shard x rows (8192 tokens -> 1024/core) and pre-transpose each shard to
    a plane-permuted [IN, M] layout (i' = k*(IN/8) + r for nibble plane k,
    qweight row r) so the device-side nibble unpack produces contraction rows in
    the matching order. Quantized weights are replicated to all cores (~9 MB).
  - Device, per 512-column output chunk: unpack int4 planes with dual-op
    tensor_scalar (shift+and) on DVE, cast on ScalarE, then W = nib*s_exp -
    zs_exp on DVE. zs_exp ([128, 4, 512], the group-expanded z*s) is built with
    4 tiny indicator matmuls gindT.T @ zs on the PE, software-pipelined one
    chunk ahead so it never serializes the dequant behind the matmul phase.
    Main accumulation out[m,o] = sum_j xt_j.T @ W_j over 32 K-blocks per PSUM
    bank; bias is added during the PSUM->SBUF eviction. The first chunk
    interleaves three token-block accumulations so the PE has work while the
    xt DMA stream lands.
  - Host: concatenate the 8 [1024, 4096] row-shards.
"""
import numpy as np

import concourse.bass as bass
import concourse.tile as tile
from concourse import bacc, mybir
from concourse.bass_utils import run_bass_kernel_spmd

N_CORES = 8
B, S, IN, OUT = 4, 2048, 4096, 4096
GROUP_SIZE = 128
M_TOT = B * S                 # 8192 tokens
M = M_TOT // N_CORES          # 1024 tokens per core
OC = 512                      # output-column chunk (one PSUM bank)

f16 = mybir.dt.float16
f32 = mybir.dt.float32
i32 = mybir.dt.int32
op = mybir.AluOpType


def build_nc(m=M, in_=IN, out=OUT):
    """Build the per-core Bass program. All shapes hardcoded for the full
    problem by default; smaller values only for simulator checks."""
    g = in_ // GROUP_SIZE
    r = in_ // 8
    nb = r // 128
    nj = in_ // 128
    noc = out // OC
    nmb = m // 128

    nc = bacc.Bacc("TRN2", target_bir_lowering=False, debug=False)

    xt_d = nc.dram_tensor("xt", [in_, m], f16, kind="ExternalInput")
    qw_d = nc.dram_tensor("qw", [r, out], i32, kind="ExternalInput")
    qz_d = nc.dram_tensor("qz", [g, out // 8], i32, kind="ExternalInput")
    sc_d = nc.dram_tensor("sc", [g, out], f16, kind="ExternalInput")
    sexp_d = nc.dram_tensor("sexp", [r, out], f16, kind="ExternalInput")
    bias_d = nc.dram_tensor("biasbc", [128, out], f16, kind="ExternalInput")
    gindt_d = nc.dram_tensor("gindt", [nb * g, 128], f16, kind="ExternalInput")
    out_d = nc.dram_tensor("out", [m, out], f16, kind="ExternalOutput")

    with tile.TileContext(nc) as tc:
        with (
            tc.tile_pool(name="persist", bufs=1) as pp,
            tc.tile_pool(name="work", bufs=1) as wp,
            tc.tile_pool(name="psum", bufs=1, space="PSUM") as psp,
        ):
            # ---- resident inputs -------------------------------------------
            xt3 = pp.tile([128, nj, m], f16)
            gindt3 = pp.tile([g, nb, 128], f16)
            nc.sync.dma_start(gindt3[:],
                              gindt_d.rearrange("(b g) p -> g b p", g=g))
            qz_sb = pp.tile([g, out // 8], i32)
            nc.sync.dma_start(qz_sb[:], qz_d[:])

            def emit_zs_dve(cc):
                """DVE/ACT part of the zero-point pipeline for chunk cc."""
                osl = slice(cc * OC, (cc + 1) * OC)
                z_ic = wp.tile([g, OC], i32, tag="z_ic", bufs=2,
                               name=f"z_ic{cc}")
                for k2 in range(8):
                    nc.vector.tensor_scalar(
                        out=z_ic[:, k2::8],
                        in0=qz_sb[:, cc * (OC // 8):(cc + 1) * (OC // 8)],
                        scalar1=4 * k2, scalar2=0xF,
                        op0=op.logical_shift_right, op1=op.bitwise_and)
                z_fc = wp.tile([g, OC], f16, tag="z_fc", bufs=2,
                               name=f"z_fc{cc}")
                nc.scalar.copy(z_fc[:], z_ic[:])
                scc = wp.tile([g, OC], f16, tag="scc", bufs=2, name=f"scc{cc}")
                nc.sync.dma_start(scc[:], sc_d[:, osl])
                zs_c = wp.tile([g, OC], f16, tag="zs_c", bufs=2,
                               name=f"zs_c{cc}")
                nc.vector.tensor_tensor(zs_c[:], z_fc[:], scc[:], op.mult)
                return zs_c

            def emit_zs_mm(cc, zs_c):
                """PE part: group-expand z*s to [128, nb, OC] via indicators."""
                zs_exp = wp.tile([128, nb, OC], f16, tag="zs_exp", bufs=2,
                                 name=f"zs_exp{cc}")
                for bb in range(nb):
                    ps_z = psp.tile([128, OC], f32, tag="ps", bufs=8,
                                    name=f"ps_z{cc}_{bb}")
                    nc.tensor.matmul(ps_z[:], gindt3[:, bb, :], zs_c[:],
                                     start=True, stop=True)
                    nc.scalar.copy(zs_exp[:, bb, :], ps_z[:])
                return zs_exp

            # bootstrap chunk 0's zero-point pipeline
            zs_exp_cur = emit_zs_mm(0, emit_zs_dve(0))

            # xt stream on the otherwise-idle gpsimd (SWDGE) queue: dma_start
            # issue time on the scalar queue would jam the ScalarE casts the
            # zero-point/dequant chain depends on, and the sync queue carries
            # the weight chunks. Grouped so matmuls start as blocks land.
            jg = 4
            for j0 in range(0, nj, jg):
                nc.gpsimd.dma_start(
                    xt3[:, j0:j0 + jg, :],
                    xt_d[j0 * 128:(j0 + jg) * 128, :].rearrange(
                        "(j p) m -> p j m", p=128))

            # ---- main loop over output-column chunks -----------------------
            for ocb in range(noc):
                osl = slice(ocb * OC, (ocb + 1) * OC)

                # next chunk's DVE-side zero-point work goes FIRST in DVE
                # program order so its zs_c is ready when the PE reaches the
                # pipelined expansion matmuls below
                zs_c_next = emit_zs_dve(ocb + 1) if ocb + 1 < noc else None

                biasc = wp.tile([128, OC], f16, tag="biasc", bufs=2,
                                name=f"biasc{ocb}")
                nc.sync.dma_start(biasc[:], bias_d[:, osl])
                qwc = wp.tile([128, nb, OC], i32, tag="qwc", bufs=1,
                              name=f"qwc{ocb}")
                nc.sync.dma_start(
                    qwc[:], qw_d[:, osl].rearrange("(b p) o -> p b o", p=128))
                sec_t = wp.tile([128, nb, OC], f16, tag="sec", bufs=2,
                                name=f"sec{ocb}")
                nc.sync.dma_start(
                    sec_t[:], sexp_d[:, osl].rearrange("(b p) o -> p b o", p=128))

                # dequant 4 row-blocks at a time per nibble plane:
                # w3[:, k*nb+bb, :] covers contraction rows i' = 128*(k*nb+bb)+p
                w3 = wp.tile([128, nj, OC], f16, tag="w3", bufs=2,
                             name=f"w3{ocb}")
                for k in range(8):
                    nib4 = wp.tile([128, nb, OC], i32, tag="nib4", bufs=2,
                                   name=f"nib4_{ocb}_{k}")
                    nc.vector.tensor_scalar(
                        out=nib4[:], in0=qwc[:], scalar1=4 * k, scalar2=0xF,
                        op0=op.logical_shift_right, op1=op.bitwise_and)
                    nibf4 = wp.tile([128, nb, OC], f16, tag="nibf4", bufs=2,
                                    name=f"nibf4_{ocb}_{k}")
                    nc.scalar.copy(nibf4[:], nib4[:])
                    wsl = w3[:, k * nb:(k + 1) * nb, :]
                    nc.vector.tensor_tensor(wsl, nibf4[:], sec_t[:], op.mult)
                    nc.vector.tensor_tensor(wsl, wsl, zs_exp_cur[:],
                                            op.subtract)

                def evict(ps, msl):
                    ot = wp.tile([128, OC], f16, tag="ot", bufs=6, name="ot")
                    nc.vector.scalar_tensor_tensor(
                        out=ot[:], in0=ps[:], scalar=0.0, in1=biasc[:],
                        op0=op.add, op1=op.add)
                    nc.sync.dma_start(out_d[msl, osl], ot[:])

                # first chunk: interleave 3 token-block accumulations so the
                # PE has ~3 matmuls available per arriving xt block
                lead = 3 if ocb == 0 else 1
                lead_ps = [psp.tile([128, OC], f32, tag="ps", bufs=8,
                                    name=f"ps_l{ocb}_{t}")
                           for t in range(min(lead, nmb))]
                for j in range(nj):
                    for t, pst in enumerate(lead_ps):
                        nc.tensor.matmul(
                            pst[:], xt3[:, j, t * 128:(t + 1) * 128],
                            w3[:, j, :], start=(j == 0), stop=(j == nj - 1))
                # pipelined zero-point expansion for the next chunk
                if zs_c_next is not None:
                    zs_exp_next = emit_zs_mm(ocb + 1, zs_c_next)
                for t, pst in enumerate(lead_ps):
                    evict(pst, slice(t * 128, (t + 1) * 128))

                for mb in range(len(lead_ps), nmb):
                    msl = slice(mb * 128, (mb + 1) * 128)
                    ps = psp.tile([128, OC], f32, tag="ps", bufs=8, name="ps")
                    for j in range(nj):
                        nc.tensor.matmul(ps[:], xt3[:, j, msl], w3[:, j, :],
                                         start=(j == 0), stop=(j == nj - 1))
                    evict(ps, msl)

                if zs_c_next is not None:
                    zs_exp_cur = zs_exp_next

    nc.compile()
    return nc


def shard_inputs(x, qweight, qzeros, scales, bias, m=M, in_=IN, out=OUT,
                 n_cores=N_CORES):
    """Host-side sharding / relayout (pure data movement + 0/1 indicators)."""
    g = in_ // GROUP_SIZE
    r = in_ // 8
    nb = r // 128

    x2 = np.asarray(x, dtype=np.float16).reshape(-1, in_)
    qweight = np.ascontiguousarray(np.asarray(qweight, dtype=np.int32))
    qzeros = np.ascontiguousarray(np.asarray(qzeros, dtype=np.int32))
    scales = np.ascontiguousarray(np.asarray(scales, dtype=np.float16))
    biasbc = np.ascontiguousarray(
        np.broadcast_to(np.asarray(bias, dtype=np.float16), (128, out)))
    sexp = np.ascontiguousarray(np.repeat(scales, 16, axis=0))

    # gindt[b*g + gg, p] = 1 iff gg == 8*b + p//16 (expansion indicator)
    gindt = np.zeros((nb * g, 128), dtype=np.float16)
    for bb in range(nb):
        for p in range(128):
            gindt[bb * g + 8 * bb + p // 16, p] = 1.0

    in_maps = []
    for c in range(n_cores):
        xc = x2[c * m:(c + 1) * m]                      # [m, in]
        xt = np.ascontiguousarray(
            xc.reshape(m, r, 8).transpose(2, 1, 0).reshape(in_, m))
        in_maps.append({
            "xt": xt, "qw": qweight, "qz": qzeros, "sc": scales,
            "sexp": sexp, "biasbc": biasbc, "gindt": gindt,
        })
    return in_maps


_NC_CACHE = {}


def kernel(x, qweight, qzeros, scales, bias):
    if "nc" not in _NC_CACHE:
        _NC_CACHE["nc"] = build_nc()
    nc = _NC_CACHE["nc"]
    in_maps = shard_inputs(x, qweight, qzeros, scales, bias)
    res = run_bass_kernel_spmd(nc, in_maps, list(range(N_CORES)))
    out = np.concatenate([res.results[c]["out"] for c in range(N_CORES)], axis=0)
    return out.reshape(B, S, OUT).astype(np.float16)
